# revision 6
# baseline (speedup 1.0000x reference)
"""LayerNorm-GRU Trainium2 kernel, v2.

B=64, T=512, D=256, H=512. Data-parallel over batch: 8 rows/core x 8 cores.

Phase 1: x-side projections in fp32r (full fp32 data, 1 cyc/row on PE),
         LayerNorm (bn_stats), PE-transpose to DRAM zx [12, 128, T*8]
         feature-major.
Phase 2: recurrence, feature-major, 8 batch rows per core. Per step:
         - PE: 48 bf16 matmuls (stationary weight tiles [128,128] bf16 ->
           fast-weight-load), ru tiles first then c tiles, z in PSUM.
         - bridge: z copy PSUM->SBUF on DVE, z^2 via ACT Square (parallel);
           per LN group so the ru chain starts before the c matmuls finish.
         - stats: DVE strided reduce over feature tiles, then (stats_engine
           'pe') a ones-column matmul for the cross-partition sums, a tiny
           DVE chain computing mean/var and 1/sqrt(var+eps) via the quake
           bitwise seed + one Newton step (no ACT Sqrt -> the single
           sigmoid/tanh/square/copy table set stays resident, zero table
           reloads), and a 1x128 ones matmul broadcasting the per-batch
           stats to all partitions.
         - apply/gates: DVE normalize + gate arithmetic; ACT sigmoid/tanh.
         Output h_t accumulates in SBUF, PE-transposed to row-major and
         DMA'd out every 16 steps.
"""

import os
import sys

for _p in ("/opt/trn_rl_repo", "/root/.axon_site/_ro/trn_rl_repo"):
    if os.path.isdir(_p) and _p not in sys.path:
        sys.path.insert(0, _p)

import numpy as np
import ml_dtypes
from contextlib import ExitStack

import concourse.bass as bass
import concourse.mybir as mybir
import concourse.tile as tile
from concourse import bacc
from concourse.bass import ds
from concourse.bass_utils import run_bass_kernel_spmd

F32 = mybir.dt.float32
F32R = mybir.dt.float32r
BF16 = mybir.dt.bfloat16
I32 = mybir.dt.int32
AX = mybir.AxisListType
OP = mybir.AluOpType
AF = mybir.ActivationFunctionType
RED = bass.bass_isa.ReduceOp

B, T, D, H = 64, 512, 256, 512
NCORES = 8
BL = B // NCORES          # 8 batch rows per core
H3 = 3 * H                # 1536
NT = H3 // 128            # 12 feature tiles
NRU = (2 * H) // 128      # 8 tiles in the r|u LN group
NC_ = H // 128            # 4 tiles in the c LN group
KH = H // 128             # 4 contraction chunks for the h-matmul
ROWS = T * BL             # 4096 rows (t-major: row = t*BL + b)
EPS = 1e-5

STEPS_PER_BODY = 128
BLK = 16                  # hist flush granularity
CHUNK = 64                # steps per xfeed chunk

MAGIC = 0x5F3759DF        # quake rsqrt seed constant
NEWTON_ITERS = 1

# engine for the scalar stats chain and for the gate arithmetic
CHAIN_ENGINE = "vector"   # 'pool' | 'vector'
APPLY_ENGINE = "vector"   # 'pool' | 'vector'
# cross-partition reduction/broadcast: gpsimd all-reduce vs PE matmuls
STATS_ENGINE = "pe"       # 'pool' | 'pe'


def _build_program(general_ln: bool, sim_steps=None,
                   chain_engine=CHAIN_ENGINE, apply_engine=APPLY_ENGINE,
                   newton_iters=NEWTON_ITERS, stats_engine=STATS_ENGINE):
    nc = bacc.Bacc("TRN2", target_bir_lowering=False, debug=False)

    xT_d = nc.dram_tensor("xT", [D, ROWS], F32R, kind="ExternalInput")
    wx_d = nc.dram_tensor("wx", [D, H3], F32R, kind="ExternalInput")
    whb_d = nc.dram_tensor("whb", [H, H3], BF16, kind="ExternalInput")
    whsb_d = nc.dram_tensor("whsb", [H, 2], BF16, kind="ExternalInput")
    h0t_d = nc.dram_tensor("h0t", [128, KH * BL], F32, kind="ExternalInput")
    ident_d = nc.dram_tensor("ident", [128, 128], F32, kind="ExternalInput")
    if general_ln:
        gx_d = nc.dram_tensor("gx", [128, H3], F32, kind="ExternalInput")
        bx_d = nc.dram_tensor("bx", [128, H3], F32, kind="ExternalInput")
        gh_d = nc.dram_tensor("gh", [128, NT], F32, kind="ExternalInput")
        bh_d = nc.dram_tensor("bh", [128, NT], F32, kind="ExternalInput")
    out_d = nc.dram_tensor("out", [BL, T, H], F32, kind="ExternalOutput")
    zx_d = nc.dram_tensor("zx", [NT, 128, ROWS + CHUNK * BL], F32,
                          kind="Internal")

    with tile.TileContext(nc) as tc, ExitStack() as ctx:
        const_pool = ctx.enter_context(tc.tile_pool(name="consts", bufs=1))
        whs = const_pool.tile([128, KH, H3], BF16)
        identity = const_pool.tile([128, 128], F32)
        epsc = const_pool.tile([128, 1], F32)
        h0t = const_pool.tile([128, KH, BL], F32)
        onescol = const_pool.tile([128, 1], F32)
        ones1 = const_pool.tile([1, 128], F32)
        onescl = const_pool.tile([128, 2], F32)   # 1/N per LN group
        whsums = const_pool.tile([128, KH, 2], BF16)
        nc.vector.memset(onescol[:], 1.0)
        nc.vector.memset(ones1[:], 1.0)
        nc.vector.memset(onescl[:, 0:1], 1.0 / (2 * H))
        nc.vector.memset(onescl[:, 1:2], 1.0 / H)
        nc.sync.dma_start(whsums[:],
                          whsb_d[:].rearrange("(k p) n -> p k n", p=128))
        if general_ln:
            gx = const_pool.tile([128, H3], F32)
            bx = const_pool.tile([128, H3], F32)
            gh = const_pool.tile([128, NT], F32)
            bh = const_pool.tile([128, NT], F32)

        nc.sync.dma_start(whs[:], whb_d[:].rearrange("(k p) n -> p k n", p=128))
        nc.sync.dma_start(identity[:], ident_d[:])
        nc.sync.dma_start(h0t[:], h0t_d[:].rearrange("p (k b) -> p k b", k=KH))
        nc.vector.memset(epsc[:], EPS)
        if general_ln:
            nc.sync.dma_start(gx[:], gx_d[:])
            nc.sync.dma_start(bx[:], bx_d[:])
            nc.sync.dma_start(gh[:], gh_d[:])
            nc.sync.dma_start(bh[:], bh_d[:])

        # ================= Phase 1: x-side projections =================
        with tc.tile_pool(name="p1sbuf", bufs=1) as p1pool, \
             tc.tile_pool(name="p1work", bufs=3) as p1work, \
             tc.tile_pool(name="p1z", bufs=2, space="PSUM") as p1z, \
             tc.tile_pool(name="p1t", bufs=2, space="PSUM") as p1t:
            xts = p1pool.tile([128, 2, ROWS], F32R)
            wxs = p1pool.tile([128, 2, H3], F32R)
            nc.sync.dma_start(xts[:], xT_d[:].rearrange("(k p) n -> p k n", p=128))
            nc.sync.dma_start(wxs[:], wx_d[:].rearrange("(k p) n -> p k n", p=128))

            for r in range(ROWS // 128):
                zp = p1z.tile([128, H3], F32, tag="zp")
                for k in range(2):
                    for nb in range(3):
                        nc.tensor.matmul(
                            zp[:, nb * 512:(nb + 1) * 512],
                            xts[:, k, r * 128:(r + 1) * 128],
                            wxs[:, k, nb * 512:(nb + 1) * 512],
                            start=(k == 0), stop=(k == 1),
                        )
                sixes = p1work.tile([128, 3, 6], F32, tag="sixes")
                aggr = p1work.tile([128, 2, 2], F32, tag="aggr")
                nc.vector.bn_stats(sixes[:, 0, :], zp[:, 0:512])
                nc.vector.bn_stats(sixes[:, 1, :], zp[:, 512:1024])
                nc.vector.bn_stats(sixes[:, 2, :], zp[:, 1024:1536])
                nc.vector.bn_aggr(aggr[:, 0, :], sixes[:, 0:2, :])
                nc.vector.bn_aggr(aggr[:, 1, :], sixes[:, 2, :])
                sd = p1work.tile([128, 2], F32, tag="sd")
                inv = p1work.tile([128, 2], F32, tag="inv")
                nc.scalar.activation(sd[:], aggr[:, :, 1], AF.Sqrt, bias=epsc[:])
                nc.vector.reciprocal(inv[:], sd[:])
                zln = p1work.tile([128, H3], F32, tag="zln")
                nc.vector.tensor_scalar(
                    zln[:, 0:1024], zp[:, 0:1024],
                    aggr[:, 0, 0:1], inv[:, 0:1], OP.subtract, OP.mult)
                nc.vector.tensor_scalar(
                    zln[:, 1024:1536], zp[:, 1024:1536],
                    aggr[:, 1, 0:1], inv[:, 1:2], OP.subtract, OP.mult)
                if general_ln:
                    nc.vector.tensor_mul(zln[:], zln[:], gx[:])
                    nc.vector.tensor_add(zln[:], zln[:], bx[:])
                if r % 2 == 0:
                    ztp = p1work.tile([128, NT, 2, 128], F32, tag="ztp")
                for m in range(NT):
                    tp = p1t.tile([128, 128], F32, tag="tp")
                    nc.tensor.transpose(tp[:], zln[:, m * 128:(m + 1) * 128],
                                        identity[:])
                    # DVE is the phase-1 bottleneck (bn_stats + LN apply);
                    # route most PSUM->SBUF staging copies to ACT instead.
                    if m % 4 == 3:
                        nc.vector.tensor_copy(ztp[:, m, r % 2, :], tp[:])
                    else:
                        nc.scalar.copy(ztp[:, m, r % 2, :], tp[:])
                if r % 2 == 1:
                    nc.sync.dma_start(
                        zx_d[:, :, (r - 1) * 128:(r + 1) * 128]
                        .transpose([1, 0, 2]),
                        ztp[:].rearrange("p t two n -> p t (two n)"))

        # ================= Phase 2: recurrence =================
        xfA = const_pool.tile([128, NT, CHUNK * BL], F32)
        xfB = const_pool.tile([128, NT, CHUNK * BL], F32)
        histP = const_pool.tile([128, KH, BLK, BL], F32)
        histQ = const_pool.tile([128, KH, BLK, BL], F32)
        obuf = const_pool.tile([128, KH, 128], F32)

        nc.vector.tensor_copy(histQ[:, :, BLK - 1, :], h0t[:])
        nc.sync.dma_start(
            xfA[:], zx_d[:, :, 0:CHUNK * BL].transpose([1, 0, 2]))

        zpool = ctx.enter_context(tc.tile_pool(name="zp2", bufs=2, space="PSUM"))
        spool = ctx.enter_context(tc.tile_pool(name="sp2", bufs=2, space="PSUM"))
        tpool = ctx.enter_context(tc.tile_pool(name="tp2", bufs=2, space="PSUM"))
        wpool = ctx.enter_context(tc.tile_pool(name="w2", bufs=3))
        hpool = ctx.enter_context(tc.tile_pool(name="hb2", bufs=3))

        ceng = {"pool": nc.gpsimd, "vector": nc.vector}[chain_engine]
        aeng = {"pool": nc.gpsimd, "vector": nc.vector}[apply_engine]

        def chain_ops(P, src_sums, n_feat, g, sb=None, goff=0):
            """Mean/var/quake-rsqrt on [P, BL] tiles from src_sums
            ([P, 2, BL]: z-sums | sq-sums). Returns (y_ap, mis_ap) as
            [P, BL] APs (for 'pe', written into SBUF stats tile)."""
            mm = wpool.tile([P, BL], F32, tag=f"mm{g}")
            ceng.tensor_scalar(mm[:], src_sums[:, 0, :], 1.0 / n_feat, None,
                               OP.mult)
            msq = wpool.tile([P, BL], F32, tag=f"msq{g}")
            ceng.tensor_tensor(msq[:], mm[:], mm[:], OP.mult)
            ve = wpool.tile([P, BL], F32, tag=f"ve{g}")
            ceng.tensor_scalar(ve[:], src_sums[:, 1, :], 1.0 / n_feat, EPS,
                               OP.mult, OP.add)
            v = wpool.tile([P, BL], F32, tag=f"v{g}")
            ceng.tensor_tensor(v[:], ve[:], msq[:], OP.subtract)
            # quake seed: one fused DVE op computes ~(i >> 1) (bitwise ops
            # are illegal on Pool); then an int add gives MAGIC - (i >> 1).
            nt_ = wpool.tile([P, BL], I32, tag=f"nt{g}")
            nc.vector.tensor_scalar(nt_[:], v[:].bitcast(I32), 1, -1,
                                    OP.logical_shift_right, OP.bitwise_xor)
            y_t = wpool.tile([P, BL], F32, tag=f"y{g}")
            y = y_t[:]
            ceng.tensor_scalar(y.bitcast(I32), nt_[:], MAGIC + 1, None,
                               OP.add)
            for it in range(newton_iters):
                a = wpool.tile([P, BL], F32, tag=f"qa{g}_{it}")
                ceng.tensor_tensor(a[:], y, y, OP.mult)
                w_ = wpool.tile([P, BL], F32, tag=f"qw{g}_{it}")
                ceng.tensor_tensor(w_[:], v[:], a[:], OP.mult)
                f_ = wpool.tile([P, BL], F32, tag=f"qf{g}_{it}")
                ceng.tensor_scalar(f_[:], w_[:], -0.5, 1.5, OP.mult, OP.add)
                last = it == newton_iters - 1
                if last and sb is not None:
                    y2 = sb[0:1, 0:BL]
                else:
                    y2_t = wpool.tile([P, BL], F32, tag=f"qy{g}_{it}")
                    y2 = y2_t[:]
                ceng.tensor_tensor(y2, y, f_[:], OP.mult)
                y = y2
            if sb is not None:
                mis = sb[0:1, BL:2 * BL]
            else:
                mis_t = wpool.tile([P, BL], F32, tag=f"mis{g}")
                mis = mis_t[:]
            ceng.tensor_tensor(mis, mm[:], y, OP.mult)
            return y, mis

        def group_chain(g, gi, zq, n_feat, ntiles, sbp):
            """Stats for one LN group. zq: SBUF [128, 2, ntiles, BL]
            (z | z^2). Returns (y_bc, mis_bc) as [128, BL] APs replicated
            on all partitions (SBUF for 'pool', PSUM for 'pe')."""
            if stats_engine == "pool":
                ps = wpool.tile([128, 2, BL], F32, tag=f"ps{g}")
                nc.vector.tensor_reduce(
                    ps[:], zq[:].rearrange("p c t b -> p c b t"), AX.X, OP.add)
                allr = wpool.tile([128, 2, BL], F32, tag=f"allr{g}")
                nc.gpsimd.partition_all_reduce(
                    allr[:].rearrange("p c b -> p (c b)"),
                    ps[:].rearrange("p c b -> p (c b)"),
                    channels=128, reduce_op=RED.add)
                y, mis = chain_ops(128, allr, n_feat, g)
                return y, mis
            # 'pe': the group mean is already accumulating in
            # sbp[0:1, gi*BL:(gi+1)*BL] via the pre-scaled folded weight
            # columns (part of the PE matmul phase); only sum(z^2) needs the
            # reduce + ones-matmul (the ones column is pre-scaled by 1/N).
            psq = wpool.tile([128, BL], F32, tag=f"ps{g}")
            nc.vector.tensor_reduce(
                psq[:], zq[:, 1, :, :].rearrange("p t b -> p b t"),
                AX.X, OP.add)
            nc.tensor.matmul(
                sbp[0:1, (2 + gi) * BL:(3 + gi) * BL], onescl[:, gi:gi + 1],
                psq[:], start=True, stop=True)
            mcp = wpool.tile([1, BL], F32, tag=f"mcp{g}")
            nc.vector.tensor_copy(mcp[:], sbp[0:1, gi * BL:(gi + 1) * BL])
            msq = wpool.tile([1, BL], F32, tag=f"msq{g}")
            ceng.tensor_tensor(msq[:], mcp[:], mcp[:], OP.mult)
            v = wpool.tile([1, BL], F32, tag=f"v{g}")
            nc.vector.scalar_tensor_tensor(
                v[:], sbp[0:1, (2 + gi) * BL:(3 + gi) * BL], EPS, msq[:],
                OP.add, OP.subtract)
            nt_ = wpool.tile([1, BL], I32, tag=f"nt{g}")
            nc.vector.tensor_scalar(nt_[:], v[:].bitcast(I32), 1, -1,
                                    OP.logical_shift_right, OP.bitwise_xor)
            st = wpool.tile([1, 2 * BL], F32, tag=f"st{g}")
            y_t = wpool.tile([1, BL], F32, tag=f"yq{g}")
            y = y_t[:]
            ceng.tensor_scalar(y.bitcast(I32), nt_[:], MAGIC + 1, None,
                               OP.add)
            for it in range(newton_iters):
                a = wpool.tile([1, BL], F32, tag=f"qa{g}_{it}")
                ceng.tensor_tensor(a[:], y, y, OP.mult)
                f_ = wpool.tile([1, BL], F32, tag=f"qf{g}_{it}")
                nc.vector.scalar_tensor_tensor(f_[:], a[:], -0.5, v[:],
                                               OP.mult, OP.mult)
                y2 = (st[0:1, 0:BL] if it == newton_iters - 1
                      else None)
                if y2 is None:
                    y2_t = wpool.tile([1, BL], F32, tag=f"qy{g}_{it}")
                    y2 = y2_t[:]
                nc.vector.scalar_tensor_tensor(y2, f_[:], 1.5, y,
                                               OP.add, OP.mult)
                y = y2
            ceng.tensor_tensor(st[0:1, BL:2 * BL], mcp[:], y, OP.mult)
            goff = (4 + 2 * gi) * BL
            nc.tensor.matmul(
                sbp[:, goff:goff + 2 * BL], ones1[0:1, :], st[0:1, :],
                start=True, stop=True)
            return (sbp[:, goff:goff + BL],
                    sbp[:, goff + BL:goff + 2 * BL])

        def emit_step(h_prev, h_out, hb_prev, xf, cstep):
            """One GRU step. h_prev/h_out: [128, KH, BL] APs (feature-major).
            hb_prev: [128, KH, BL] bf16 tile; returns the next hb tile."""
            zru = zpool.tile([128, NRU, BL], F32, tag="zru")
            zc = zpool.tile([128, NC_, BL], F32, tag="zc")
            sbp = None
            if stats_engine == "pe":
                sbp = spool.tile([128, 8 * BL], F32, tag="sb")
            for m in range(NRU):
                for k in range(KH):
                    nc.tensor.matmul(
                        zru[:, m, :], whs[:, k, m * 128:(m + 1) * 128],
                        hb_prev[:, k, :], start=(k == 0), stop=(k == KH - 1))
            if stats_engine == "pe":
                # group means ride along as two extra matmul columns against
                # the pre-scaled folded weight sums
                for gi in range(2):
                    for k in range(KH):
                        nc.tensor.matmul(
                            sbp[0:1, gi * BL:(gi + 1) * BL],
                            whsums[:, k, gi:gi + 1], hb_prev[:, k, :],
                            start=(k == 0), stop=(k == KH - 1))
            for m in range(NC_):
                for k in range(KH):
                    nc.tensor.matmul(
                        zc[:, m, :], whs[:, k, (NRU + m) * 128:(NRU + m + 1) * 128],
                        hb_prev[:, k, :], start=(k == 0), stop=(k == KH - 1))

            # bridge PSUM -> SBUF: z copy on DVE, square on ACT (parallel)
            zqru = wpool.tile([128, 2, NRU, BL], F32, tag="zqru")
            nc.scalar.activation(
                zqru[:, 1, :, :].rearrange("p t b -> p (t b)"),
                zru[:].rearrange("p t b -> p (t b)"), AF.Square)
            zqc = wpool.tile([128, 2, NC_, BL], F32, tag="zqc")
            nc.scalar.activation(
                zqc[:, 1, :, :].rearrange("p t b -> p (t b)"),
                zc[:].rearrange("p t b -> p (t b)"), AF.Square)
            # z copies ride the ACT slack behind the critical squares
            nc.scalar.copy(
                zqru[:, 0, :, :].rearrange("p t b -> p (t b)"),
                zru[:].rearrange("p t b -> p (t b)"))
            nc.scalar.copy(
                zqc[:, 0, :, :].rearrange("p t b -> p (t b)"),
                zc[:].rearrange("p t b -> p (t b)"))

            y_ru, mis_ru = group_chain("r", 0, zqru, 2.0 * H, NRU, sbp)

            xs = xf[:, :, cstep * BL:(cstep + 1) * BL]
            # ru apply: pre = z*is + (x - mis)  (emitted BEFORE the c-group
            # chain so the scheduler runs the c chain during sigmoid, not
            # ahead of the critical ru-apply path)
            xm = wpool.tile([128, NRU, BL], F32, tag="xm")
            aeng.tensor_tensor(
                xm[:], xs[:, 0:NRU, :],
                mis_ru.unsqueeze(1).to_broadcast([128, NRU, BL]),
                OP.subtract)
            tru = wpool.tile([128, NRU, BL], F32, tag="tru")
            aeng.tensor_tensor(
                tru[:], zqru[:, 0, :, :],
                y_ru.unsqueeze(1).to_broadcast([128, NRU, BL]), OP.mult)
            if general_ln:
                nc.vector.tensor_mul(
                    tru[:], tru[:],
                    gh[:, 0:NRU].unsqueeze(2).to_broadcast([128, NRU, BL]))
                gmis = wpool.tile([128, NRU, BL], F32, tag="gmis")
                nc.vector.tensor_tensor(
                    gmis[:],
                    mis_ru.unsqueeze(1).to_broadcast([128, NRU, BL]),
                    gh[:, 0:NRU].unsqueeze(2).to_broadcast([128, NRU, BL]),
                    OP.mult)
                nc.vector.tensor_tensor(
                    xm[:], xs[:, 0:NRU, :], gmis[:], OP.subtract)
                nc.vector.tensor_add(
                    xm[:], xm[:],
                    bh[:, 0:NRU].unsqueeze(2).to_broadcast([128, NRU, BL]))
            pre = wpool.tile([128, NRU, BL], F32, tag="pre")
            aeng.tensor_tensor(pre[:], tru[:], xm[:], OP.add)
            sig = wpool.tile([128, NRU, BL], F32, tag="sig")
            nc.scalar.activation(
                sig[:].rearrange("p a b -> p (a b)"),
                pre[:].rearrange("p a b -> p (a b)"), AF.Sigmoid)

            y_c, mis_c = group_chain("c", 1, zqc, float(H), NC_, sbp)
            # c apply
            tc_ = wpool.tile([128, NC_, BL], F32, tag="tc_")
            aeng.tensor_tensor(
                tc_[:], zqc[:, 0, :, :],
                y_c.unsqueeze(1).to_broadcast([128, NC_, BL]), OP.mult)
            oc = wpool.tile([128, NC_, BL], F32, tag="oc")
            aeng.tensor_tensor(
                oc[:], tc_[:],
                mis_c.unsqueeze(1).to_broadcast([128, NC_, BL]),
                OP.subtract)
            if general_ln:
                nc.vector.tensor_mul(
                    oc[:], oc[:],
                    gh[:, NRU:NT].unsqueeze(2).to_broadcast([128, NC_, BL]))
                nc.vector.tensor_add(
                    oc[:], oc[:],
                    bh[:, NRU:NT].unsqueeze(2).to_broadcast([128, NC_, BL]))
            rh = wpool.tile([128, NC_, BL], F32, tag="rh")
            aeng.tensor_tensor(rh[:], sig[:, 0:NC_, :], oc[:], OP.mult)
            prec = wpool.tile([128, NC_, BL], F32, tag="prec")
            aeng.tensor_tensor(prec[:], rh[:], xs[:, NRU:NT, :], OP.add)
            cc = wpool.tile([128, NC_, BL], F32, tag="cc")
            nc.scalar.activation(
                cc[:].rearrange("p a b -> p (a b)"),
                prec[:].rearrange("p a b -> p (a b)"), AF.Tanh)
            dd = wpool.tile([128, KH, BL], F32, tag="dd")
            aeng.tensor_tensor(dd[:], cc[:], h_prev, OP.subtract)
            ud = wpool.tile([128, KH, BL], F32, tag="ud")
            aeng.tensor_tensor(ud[:], sig[:, NC_:NRU, :], dd[:], OP.mult)
            # bf16 h for the next step's matmuls FIRST (it gates the PE),
            # then the fp32 hist/output copy off the critical path
            hb = hpool.tile([128, KH, BL], BF16, tag="hb")
            aeng.tensor_tensor(hb[:], h_prev, ud[:], OP.add)
            aeng.tensor_tensor(h_out, h_prev, ud[:], OP.add)
            return hb

        def flush_block(hist, tb_expr):
            for k in range(KH):
                tp = tpool.tile([128, 128], F32, tag="ftp")
                nc.tensor.transpose(tp[:], hist[:, k, :, :], identity[:])
                if k % 2 == 0:
                    nc.scalar.copy(obuf[:, k, :], tp[:])
                else:
                    nc.vector.tensor_copy(obuf[:, k, :], tp[:])
            nc.sync.dma_start(
                out_d[:, ds(tb_expr, BLK), :].transpose([1, 0, 2]),
                obuf[:].rearrange("p k n -> p (k n)"))

        def _emit_body(ib):
            hb = hpool.tile([128, KH, BL], BF16, tag="hb")
            nc.vector.tensor_copy(hb[:], histQ[:, :, BLK - 1, :])
            nc.sync.dma_start(
                xfB[:],
                zx_d[:, :, ds((ib + CHUNK) * BL, CHUNK * BL)].transpose([1, 0, 2]))
            for half in range(2):
                xf = (xfA, xfB)[half]
                for blk in range(4):
                    gblk = half * 4 + blk
                    hist = (histP, histQ)[gblk % 2]
                    prev_hist = (histP, histQ)[(gblk + 1) % 2]
                    for s in range(BLK):
                        cstep = blk * BLK + s
                        h_prev = (hist[:, :, s - 1, :] if s > 0
                                  else prev_hist[:, :, BLK - 1, :])
                        hb = emit_step(h_prev, hist[:, :, s, :], hb, xf, cstep)
                    flush_block(hist, ib + gblk * BLK)
            nc.sync.dma_start(
                xfA[:],
                zx_d[:, :, ds((ib + 2 * CHUNK) * BL, CHUNK * BL)].transpose([1, 0, 2]))

        if sim_steps is not None:
            for ib2 in range(0, sim_steps, STEPS_PER_BODY):
                _emit_body(ib2)
        else:
            with tc.For_i(0, T, STEPS_PER_BODY,
                          hint_engines=(mybir.EngineType.PE,
                                        mybir.EngineType.DVE,
                                        mybir.EngineType.Activation,
                                        mybir.EngineType.Pool)) as ib:
                _emit_body(ib)

    nc.compile()
    return nc


_CACHE = {}
LAST_RESULT = None


def _get_program(general_ln: bool):
    if general_ln not in _CACHE:
        _CACHE[general_ln] = _build_program(general_ln)
    return _CACHE[general_ln]


def build_in_maps(inputs):
    return _prep(**inputs)[0]


def _prep(x, W_xr, W_xu, W_xc, W_hr, W_hu, W_hc, h0,
          ln_xru_scale, ln_xru_bias, ln_hru_scale, ln_hru_bias,
          ln_xc_scale, ln_xc_bias, ln_hc_scale, ln_hc_bias):
    x = np.ascontiguousarray(np.asarray(x, np.float32))
    wx = np.concatenate([W_xr, W_xu, W_xc], axis=1).astype(np.float32)
    wh = np.concatenate([W_hr, W_hu, W_hc], axis=1).astype(np.float32)
    whb = np.ascontiguousarray(wh.astype(ml_dtypes.bfloat16))
    whf = whb.astype(np.float32)
    whsb = np.stack([whf[:, :2 * H].sum(1) / (2 * H),
                     whf[:, 2 * H:].sum(1) / H], axis=1)
    whsb = np.ascontiguousarray(whsb.astype(ml_dtypes.bfloat16))

    gx_full = np.concatenate([ln_xru_scale, ln_xc_scale]).astype(np.float32)
    bx_full = np.concatenate([ln_xru_bias, ln_xc_bias]).astype(np.float32)
    gh_full = np.concatenate([ln_hru_scale, ln_hc_scale]).astype(np.float32)
    bh_full = np.concatenate([ln_hru_bias, ln_hc_bias]).astype(np.float32)
    general_ln = not (np.all(gx_full == 1) and np.all(bx_full == 0)
                      and np.all(gh_full == 1) and np.all(bh_full == 0))

    h0 = np.asarray(h0, np.float32)
    h0t = np.repeat(h0.reshape(KH, 128).T[:, :, None], BL, axis=2)
    h0t = np.ascontiguousarray(h0t.reshape(128, KH * BL), np.float32)

    ident = np.eye(128, dtype=np.float32)

    shared = {
        "wx": np.ascontiguousarray(wx), "whb": whb, "whsb": whsb,
        "h0t": h0t, "ident": ident,
    }
    if general_ln:
        shared["gx"] = np.broadcast_to(gx_full, (128, H3)).copy()
        shared["bx"] = np.broadcast_to(bx_full, (128, H3)).copy()
        shared["gh"] = np.ascontiguousarray(gh_full.reshape(NT, 128).T)
        shared["bh"] = np.ascontiguousarray(bh_full.reshape(NT, 128).T)

    in_maps = []
    for c in range(NCORES):
        xl = x[c * BL:(c + 1) * BL]                      # [BL, T, D]
        xT = np.ascontiguousarray(
            xl.transpose(2, 1, 0).reshape(D, ROWS), np.float32)
        in_maps.append({"xT": xT, **shared})

    return in_maps, general_ln


def kernel(**inputs):
    in_maps, general_ln = _prep(**inputs)
    nc = _get_program(general_ln)
    res = run_bass_kernel_spmd(nc, in_maps, list(range(NCORES)))
    global LAST_RESULT
    LAST_RESULT = res
    outs = [res.results[c]["out"] for c in range(NCORES)]
    return np.concatenate(outs, axis=0).astype(np.float32)


if __name__ == "__main__":
    rng = np.random.default_rng(0)
    ins = {
        "x": rng.standard_normal((B, T, D), dtype=np.float32),
        "W_xr": rng.standard_normal((D, H), dtype=np.float32) / np.sqrt(D),
        "W_xu": rng.standard_normal((D, H), dtype=np.float32) / np.sqrt(D),
        "W_xc": rng.standard_normal((D, H), dtype=np.float32) / np.sqrt(D),
        "W_hr": rng.standard_normal((H, H), dtype=np.float32) / np.sqrt(H),
        "W_hu": rng.standard_normal((H, H), dtype=np.float32) / np.sqrt(H),
        "W_hc": rng.standard_normal((H, H), dtype=np.float32) / np.sqrt(H),
        "h0": np.zeros(H, np.float32),
        "ln_xru_scale": np.ones(2 * H, np.float32),
        "ln_xru_bias": np.zeros(2 * H, np.float32),
        "ln_hru_scale": np.ones(2 * H, np.float32),
        "ln_hru_bias": np.zeros(2 * H, np.float32),
        "ln_xc_scale": np.ones(H, np.float32),
        "ln_xc_bias": np.zeros(H, np.float32),
        "ln_hc_scale": np.ones(H, np.float32),
        "ln_hc_bias": np.zeros(H, np.float32),
    }
    out = kernel(**ins)
    print(out.shape, out.dtype, np.abs(out).mean())


# revision 7
# speedup vs baseline: 1.0121x; 1.0121x over previous
"""LayerNorm-GRU Trainium2 kernel, v2.

B=64, T=512, D=256, H=512. Data-parallel over batch: 8 rows/core x 8 cores.

Phase 1: x-side projections in fp32r (full fp32 data, 1 cyc/row on PE),
         LayerNorm (bn_stats), PE-transpose to DRAM zx [12, 128, T*8]
         feature-major.
Phase 2: recurrence, feature-major, 8 batch rows per core. Per step:
         - PE: 48 bf16 matmuls (stationary weight tiles [128,128] bf16 ->
           fast-weight-load), ru tiles first then c tiles, z in PSUM.
         - bridge: z copy PSUM->SBUF on DVE, z^2 via ACT Square (parallel);
           per LN group so the ru chain starts before the c matmuls finish.
         - stats: DVE strided reduce over feature tiles, then (stats_engine
           'pe') a ones-column matmul for the cross-partition sums, a tiny
           DVE chain computing mean/var and 1/sqrt(var+eps) via the quake
           bitwise seed + one Newton step (no ACT Sqrt -> the single
           sigmoid/tanh/square/copy table set stays resident, zero table
           reloads), and a 1x128 ones matmul broadcasting the per-batch
           stats to all partitions.
         - apply/gates: DVE normalize + gate arithmetic; ACT sigmoid/tanh.
         Output h_t accumulates in SBUF, PE-transposed to row-major and
         DMA'd out every 16 steps.
"""

import os
import sys

for _p in ("/opt/trn_rl_repo", "/root/.axon_site/_ro/trn_rl_repo"):
    if os.path.isdir(_p) and _p not in sys.path:
        sys.path.insert(0, _p)

import numpy as np
import ml_dtypes
from contextlib import ExitStack

import concourse.bass as bass
import concourse.mybir as mybir
import concourse.tile as tile
from concourse import bacc
from concourse.bass import ds
from concourse.bass_utils import run_bass_kernel_spmd

F32 = mybir.dt.float32
F32R = mybir.dt.float32r
BF16 = mybir.dt.bfloat16
I32 = mybir.dt.int32
AX = mybir.AxisListType
OP = mybir.AluOpType
AF = mybir.ActivationFunctionType
RED = bass.bass_isa.ReduceOp

B, T, D, H = 64, 512, 256, 512
NCORES = 8
BL = B // NCORES          # 8 batch rows per core
H3 = 3 * H                # 1536
NT = H3 // 128            # 12 feature tiles
NRU = (2 * H) // 128      # 8 tiles in the r|u LN group
NC_ = H // 128            # 4 tiles in the c LN group
KH = H // 128             # 4 contraction chunks for the h-matmul
ROWS = T * BL             # 4096 rows (t-major: row = t*BL + b)
EPS = 1e-5

STEPS_PER_BODY = 128
BLK = 16                  # hist flush granularity
CHUNK = 64                # steps per xfeed chunk

MAGIC = 0x5F3759DF        # quake rsqrt seed constant
NEWTON_ITERS = 1

# engine for the scalar stats chain and for the gate arithmetic
CHAIN_ENGINE = "vector"   # 'pool' | 'vector'
APPLY_ENGINE = "vector"   # 'pool' | 'vector'
# cross-partition reduction/broadcast: gpsimd all-reduce vs PE matmuls
STATS_ENGINE = "pe"       # 'pool' | 'pe'


def _build_program(general_ln: bool, sim_steps=None,
                   chain_engine=CHAIN_ENGINE, apply_engine=APPLY_ENGINE,
                   newton_iters=NEWTON_ITERS, stats_engine=STATS_ENGINE):
    nc = bacc.Bacc("TRN2", target_bir_lowering=False, debug=False)

    xT_d = nc.dram_tensor("xT", [D, ROWS], F32R, kind="ExternalInput")
    wx_d = nc.dram_tensor("wx", [D, H3], F32R, kind="ExternalInput")
    whb_d = nc.dram_tensor("whb", [H, H3], BF16, kind="ExternalInput")
    whsb_d = nc.dram_tensor("whsb", [H, 2], BF16, kind="ExternalInput")
    h0t_d = nc.dram_tensor("h0t", [128, KH * BL], F32, kind="ExternalInput")
    ident_d = nc.dram_tensor("ident", [128, 128], F32, kind="ExternalInput")
    if general_ln:
        gx_d = nc.dram_tensor("gx", [128, H3], F32, kind="ExternalInput")
        bx_d = nc.dram_tensor("bx", [128, H3], F32, kind="ExternalInput")
        gh_d = nc.dram_tensor("gh", [128, NT], F32, kind="ExternalInput")
        bh_d = nc.dram_tensor("bh", [128, NT], F32, kind="ExternalInput")
    out_d = nc.dram_tensor("out", [BL, T, H], F32, kind="ExternalOutput")
    zx_d = nc.dram_tensor("zx", [NT, 128, ROWS + CHUNK * BL], F32,
                          kind="Internal")

    with tile.TileContext(nc) as tc, ExitStack() as ctx:
        const_pool = ctx.enter_context(tc.tile_pool(name="consts", bufs=1))
        whs = const_pool.tile([128, KH, H3], BF16)
        identity = const_pool.tile([128, 128], F32)
        epsc = const_pool.tile([128, 1], F32)
        h0t = const_pool.tile([128, KH, BL], F32)
        onescol = const_pool.tile([128, 1], F32)
        ones1 = const_pool.tile([1, 128], F32)
        onescl = const_pool.tile([128, 2], F32)   # 1/N per LN group
        whsums = const_pool.tile([128, KH, 2], BF16)
        nc.vector.memset(onescol[:], 1.0)
        nc.vector.memset(ones1[:], 1.0)
        nc.vector.memset(onescl[:, 0:1], 1.0 / (2 * H))
        nc.vector.memset(onescl[:, 1:2], 1.0 / H)
        nc.sync.dma_start(whsums[:],
                          whsb_d[:].rearrange("(k p) n -> p k n", p=128))
        if general_ln:
            gx = const_pool.tile([128, H3], F32)
            bx = const_pool.tile([128, H3], F32)
            gh = const_pool.tile([128, NT], F32)
            bh = const_pool.tile([128, NT], F32)

        nc.sync.dma_start(whs[:], whb_d[:].rearrange("(k p) n -> p k n", p=128))
        nc.sync.dma_start(identity[:], ident_d[:])
        nc.sync.dma_start(h0t[:], h0t_d[:].rearrange("p (k b) -> p k b", k=KH))
        nc.vector.memset(epsc[:], EPS)
        if general_ln:
            nc.sync.dma_start(gx[:], gx_d[:])
            nc.sync.dma_start(bx[:], bx_d[:])
            nc.sync.dma_start(gh[:], gh_d[:])
            nc.sync.dma_start(bh[:], bh_d[:])

        # ================= Phase 1: x-side projections =================
        with tc.tile_pool(name="p1sbuf", bufs=1) as p1pool, \
             tc.tile_pool(name="p1work", bufs=3) as p1work, \
             tc.tile_pool(name="p1z", bufs=2, space="PSUM") as p1z, \
             tc.tile_pool(name="p1t", bufs=2, space="PSUM") as p1t:
            xts = p1pool.tile([128, 2, ROWS], F32R)
            wxs = p1pool.tile([128, 2, H3], F32R)
            nc.sync.dma_start(xts[:], xT_d[:].rearrange("(k p) n -> p k n", p=128))
            nc.sync.dma_start(wxs[:], wx_d[:].rearrange("(k p) n -> p k n", p=128))

            for r in range(ROWS // 128):
                zp = p1z.tile([128, H3], F32, tag="zp")
                for k in range(2):
                    for nb in range(3):
                        nc.tensor.matmul(
                            zp[:, nb * 512:(nb + 1) * 512],
                            xts[:, k, r * 128:(r + 1) * 128],
                            wxs[:, k, nb * 512:(nb + 1) * 512],
                            start=(k == 0), stop=(k == 1),
                        )
                sixes = p1work.tile([128, 3, 6], F32, tag="sixes")
                aggr = p1work.tile([128, 2, 2], F32, tag="aggr")
                nc.vector.bn_stats(sixes[:, 0, :], zp[:, 0:512])
                nc.vector.bn_stats(sixes[:, 1, :], zp[:, 512:1024])
                nc.vector.bn_stats(sixes[:, 2, :], zp[:, 1024:1536])
                nc.vector.bn_aggr(aggr[:, 0, :], sixes[:, 0:2, :])
                nc.vector.bn_aggr(aggr[:, 1, :], sixes[:, 2, :])
                sd = p1work.tile([128, 2], F32, tag="sd")
                inv = p1work.tile([128, 2], F32, tag="inv")
                nc.scalar.activation(sd[:], aggr[:, :, 1], AF.Sqrt, bias=epsc[:])
                nc.vector.reciprocal(inv[:], sd[:])
                zln = p1work.tile([128, H3], F32, tag="zln")
                nc.vector.tensor_scalar(
                    zln[:, 0:1024], zp[:, 0:1024],
                    aggr[:, 0, 0:1], inv[:, 0:1], OP.subtract, OP.mult)
                nc.vector.tensor_scalar(
                    zln[:, 1024:1536], zp[:, 1024:1536],
                    aggr[:, 1, 0:1], inv[:, 1:2], OP.subtract, OP.mult)
                if general_ln:
                    nc.vector.tensor_mul(zln[:], zln[:], gx[:])
                    nc.vector.tensor_add(zln[:], zln[:], bx[:])
                if r % 2 == 0:
                    ztp = p1work.tile([128, NT, 2, 128], F32, tag="ztp")
                for m in range(NT):
                    tp = p1t.tile([128, 128], F32, tag="tp")
                    nc.tensor.transpose(tp[:], zln[:, m * 128:(m + 1) * 128],
                                        identity[:])
                    # DVE is the phase-1 bottleneck (bn_stats + LN apply);
                    # route most PSUM->SBUF staging copies to ACT instead.
                    if m % 4 == 3:
                        nc.vector.tensor_copy(ztp[:, m, r % 2, :], tp[:])
                    else:
                        nc.scalar.copy(ztp[:, m, r % 2, :], tp[:])
                if r % 2 == 1:
                    nc.sync.dma_start(
                        zx_d[:, :, (r - 1) * 128:(r + 1) * 128]
                        .transpose([1, 0, 2]),
                        ztp[:].rearrange("p t two n -> p t (two n)"))

        # ================= Phase 2: recurrence =================
        xfA = const_pool.tile([128, NT, CHUNK * BL], F32)
        xfB = const_pool.tile([128, NT, CHUNK * BL], F32)
        histP = const_pool.tile([128, KH, BLK, BL], F32)
        histQ = const_pool.tile([128, KH, BLK, BL], F32)
        obuf = const_pool.tile([128, KH, 128], F32)

        nc.vector.tensor_copy(histQ[:, :, BLK - 1, :], h0t[:])
        nc.sync.dma_start(
            xfA[:], zx_d[:, :, 0:CHUNK * BL].transpose([1, 0, 2]))

        zpool = ctx.enter_context(tc.tile_pool(name="zp2", bufs=2, space="PSUM"))
        spool = ctx.enter_context(tc.tile_pool(name="sp2", bufs=2, space="PSUM"))
        mpool = ctx.enter_context(tc.tile_pool(name="mp2", bufs=1, space="PSUM"))
        tpool = ctx.enter_context(tc.tile_pool(name="tp2", bufs=1, space="PSUM"))
        wpool = ctx.enter_context(tc.tile_pool(name="w2", bufs=3))
        hpool = ctx.enter_context(tc.tile_pool(name="hb2", bufs=3))

        ceng = {"pool": nc.gpsimd, "vector": nc.vector}[chain_engine]
        aeng = {"pool": nc.gpsimd, "vector": nc.vector}[apply_engine]

        def chain_ops(P, src_sums, n_feat, g, sb=None, goff=0):
            """Mean/var/quake-rsqrt on [P, BL] tiles from src_sums
            ([P, 2, BL]: z-sums | sq-sums). Returns (y_ap, mis_ap) as
            [P, BL] APs (for 'pe', written into SBUF stats tile)."""
            mm = wpool.tile([P, BL], F32, tag=f"mm{g}")
            ceng.tensor_scalar(mm[:], src_sums[:, 0, :], 1.0 / n_feat, None,
                               OP.mult)
            msq = wpool.tile([P, BL], F32, tag=f"msq{g}")
            ceng.tensor_tensor(msq[:], mm[:], mm[:], OP.mult)
            ve = wpool.tile([P, BL], F32, tag=f"ve{g}")
            ceng.tensor_scalar(ve[:], src_sums[:, 1, :], 1.0 / n_feat, EPS,
                               OP.mult, OP.add)
            v = wpool.tile([P, BL], F32, tag=f"v{g}")
            ceng.tensor_tensor(v[:], ve[:], msq[:], OP.subtract)
            # quake seed: one fused DVE op computes ~(i >> 1) (bitwise ops
            # are illegal on Pool); then an int add gives MAGIC - (i >> 1).
            nt_ = wpool.tile([P, BL], I32, tag=f"nt{g}")
            nc.vector.tensor_scalar(nt_[:], v[:].bitcast(I32), 1, -1,
                                    OP.logical_shift_right, OP.bitwise_xor)
            y_t = wpool.tile([P, BL], F32, tag=f"y{g}")
            y = y_t[:]
            ceng.tensor_scalar(y.bitcast(I32), nt_[:], MAGIC + 1, None,
                               OP.add)
            for it in range(newton_iters):
                a = wpool.tile([P, BL], F32, tag=f"qa{g}_{it}")
                ceng.tensor_tensor(a[:], y, y, OP.mult)
                w_ = wpool.tile([P, BL], F32, tag=f"qw{g}_{it}")
                ceng.tensor_tensor(w_[:], v[:], a[:], OP.mult)
                f_ = wpool.tile([P, BL], F32, tag=f"qf{g}_{it}")
                ceng.tensor_scalar(f_[:], w_[:], -0.5, 1.5, OP.mult, OP.add)
                last = it == newton_iters - 1
                if last and sb is not None:
                    y2 = sb[0:1, 0:BL]
                else:
                    y2_t = wpool.tile([P, BL], F32, tag=f"qy{g}_{it}")
                    y2 = y2_t[:]
                ceng.tensor_tensor(y2, y, f_[:], OP.mult)
                y = y2
            if sb is not None:
                mis = sb[0:1, BL:2 * BL]
            else:
                mis_t = wpool.tile([P, BL], F32, tag=f"mis{g}")
                mis = mis_t[:]
            ceng.tensor_tensor(mis, mm[:], y, OP.mult)
            return y, mis

        def group_chain(g, gi, zq, n_feat, ntiles, sbp, mbp):
            """Stats for one LN group. zq: SBUF [128, 2, ntiles, BL]
            (z | z^2). Returns (y_bc, mis_bc) as [128, BL] APs replicated
            on all partitions (SBUF for 'pool', PSUM for 'pe')."""
            if stats_engine == "pool":
                ps = wpool.tile([128, 2, BL], F32, tag=f"ps{g}")
                nc.vector.tensor_reduce(
                    ps[:], zq[:].rearrange("p c t b -> p c b t"), AX.X, OP.add)
                allr = wpool.tile([128, 2, BL], F32, tag=f"allr{g}")
                nc.gpsimd.partition_all_reduce(
                    allr[:].rearrange("p c b -> p (c b)"),
                    ps[:].rearrange("p c b -> p (c b)"),
                    channels=128, reduce_op=RED.add)
                y, mis = chain_ops(128, allr, n_feat, g)
                return y, mis
            # 'pe': the group mean is already accumulating in
            # sbp[0:1, gi*BL:(gi+1)*BL] via the pre-scaled folded weight
            # columns (part of the PE matmul phase); only sum(z^2) needs the
            # reduce + ones-matmul (the ones column is pre-scaled by 1/N).
            psq = wpool.tile([128, BL], F32, tag=f"ps{g}")
            nc.vector.tensor_reduce(
                psq[:], zq[:, 1, :, :].rearrange("p t b -> p b t"),
                AX.X, OP.add)
            nc.tensor.matmul(
                sbp[0:1, gi * BL:(gi + 1) * BL], onescl[:, gi:gi + 1],
                psq[:], start=True, stop=True)
            mcp = wpool.tile([1, BL], F32, tag=f"mcp{g}")
            nc.vector.tensor_copy(mcp[:], mbp[0:1, gi * BL:(gi + 1) * BL])
            msq = wpool.tile([1, BL], F32, tag=f"msq{g}")
            ceng.tensor_tensor(msq[:], mcp[:], mcp[:], OP.mult)
            v = wpool.tile([1, BL], F32, tag=f"v{g}")
            nc.vector.scalar_tensor_tensor(
                v[:], sbp[0:1, gi * BL:(gi + 1) * BL], EPS, msq[:],
                OP.add, OP.subtract)
            nt_ = wpool.tile([1, BL], I32, tag=f"nt{g}")
            nc.vector.tensor_scalar(nt_[:], v[:].bitcast(I32), 1, -1,
                                    OP.logical_shift_right, OP.bitwise_xor)
            st = wpool.tile([1, 2 * BL], F32, tag=f"st{g}")
            y_t = wpool.tile([1, BL], F32, tag=f"yq{g}")
            y = y_t[:]
            ceng.tensor_scalar(y.bitcast(I32), nt_[:], MAGIC + 1, None,
                               OP.add)
            for it in range(newton_iters):
                a = wpool.tile([1, BL], F32, tag=f"qa{g}_{it}")
                ceng.tensor_tensor(a[:], y, y, OP.mult)
                f_ = wpool.tile([1, BL], F32, tag=f"qf{g}_{it}")
                nc.vector.scalar_tensor_tensor(f_[:], a[:], -0.5, v[:],
                                               OP.mult, OP.mult)
                y2 = (st[0:1, 0:BL] if it == newton_iters - 1
                      else None)
                if y2 is None:
                    y2_t = wpool.tile([1, BL], F32, tag=f"qy{g}_{it}")
                    y2 = y2_t[:]
                nc.vector.scalar_tensor_tensor(y2, f_[:], 1.5, y,
                                               OP.add, OP.mult)
                y = y2
            ceng.tensor_tensor(st[0:1, BL:2 * BL], mcp[:], y, OP.mult)
            goff = (2 + 2 * gi) * BL
            nc.tensor.matmul(
                sbp[:, goff:goff + 2 * BL], ones1[0:1, :], st[0:1, :],
                start=True, stop=True)
            return (sbp[:, goff:goff + BL],
                    sbp[:, goff + BL:goff + 2 * BL])

        def emit_step(h_prev, h_out, hb_prev, xf, cstep):
            """One GRU step. h_prev/h_out: [128, KH, BL] APs (feature-major).
            hb_prev: [128, KH, BL] bf16 tile; returns the next hb tile."""
            zru = zpool.tile([128, NRU, BL], F32, tag="zru")
            zc = zpool.tile([128, NC_, BL], F32, tag="zc")
            sbp = mbp = None
            if stats_engine == "pe":
                sbp = spool.tile([128, 8 * BL], F32, tag="sb")
                mbp = mpool.tile([1, 2 * BL], F32, tag="mb")
            for m in range(NRU):
                for k in range(KH):
                    nc.tensor.matmul(
                        zru[:, m, :], whs[:, k, m * 128:(m + 1) * 128],
                        hb_prev[:, k, :], start=(k == 0), stop=(k == KH - 1))
            if stats_engine == "pe":
                # group means ride along as two extra matmul columns against
                # the pre-scaled folded weight sums
                for gi in range(2):
                    for k in range(KH):
                        nc.tensor.matmul(
                            mbp[0:1, gi * BL:(gi + 1) * BL],
                            whsums[:, k, gi:gi + 1], hb_prev[:, k, :],
                            start=(k == 0), stop=(k == KH - 1))
            for m in range(NC_):
                for k in range(KH):
                    nc.tensor.matmul(
                        zc[:, m, :], whs[:, k, (NRU + m) * 128:(NRU + m + 1) * 128],
                        hb_prev[:, k, :], start=(k == 0), stop=(k == KH - 1))

            # bridge PSUM -> SBUF: z copy on DVE, square on ACT (parallel)
            zqru = wpool.tile([128, 2, NRU, BL], F32, tag="zqru")
            nc.scalar.activation(
                zqru[:, 1, :, :].rearrange("p t b -> p (t b)"),
                zru[:].rearrange("p t b -> p (t b)"), AF.Square)
            zqc = wpool.tile([128, 2, NC_, BL], F32, tag="zqc")
            nc.scalar.activation(
                zqc[:, 1, :, :].rearrange("p t b -> p (t b)"),
                zc[:].rearrange("p t b -> p (t b)"), AF.Square)
            # z copies ride the ACT slack behind the critical squares
            nc.scalar.copy(
                zqru[:, 0, :, :].rearrange("p t b -> p (t b)"),
                zru[:].rearrange("p t b -> p (t b)"))
            nc.scalar.copy(
                zqc[:, 0, :, :].rearrange("p t b -> p (t b)"),
                zc[:].rearrange("p t b -> p (t b)"))

            y_ru, mis_ru = group_chain("r", 0, zqru, 2.0 * H, NRU, sbp, mbp)

            xs = xf[:, :, cstep * BL:(cstep + 1) * BL]
            # ru apply: pre = z*is + (x - mis)  (emitted BEFORE the c-group
            # chain so the scheduler runs the c chain during sigmoid, not
            # ahead of the critical ru-apply path)
            xm = wpool.tile([128, NRU, BL], F32, tag="xm")
            aeng.tensor_tensor(
                xm[:], xs[:, 0:NRU, :],
                mis_ru.unsqueeze(1).to_broadcast([128, NRU, BL]),
                OP.subtract)
            tru = wpool.tile([128, NRU, BL], F32, tag="tru")
            aeng.tensor_tensor(
                tru[:], zqru[:, 0, :, :],
                y_ru.unsqueeze(1).to_broadcast([128, NRU, BL]), OP.mult)
            if general_ln:
                nc.vector.tensor_mul(
                    tru[:], tru[:],
                    gh[:, 0:NRU].unsqueeze(2).to_broadcast([128, NRU, BL]))
                gmis = wpool.tile([128, NRU, BL], F32, tag="gmis")
                nc.vector.tensor_tensor(
                    gmis[:],
                    mis_ru.unsqueeze(1).to_broadcast([128, NRU, BL]),
                    gh[:, 0:NRU].unsqueeze(2).to_broadcast([128, NRU, BL]),
                    OP.mult)
                nc.vector.tensor_tensor(
                    xm[:], xs[:, 0:NRU, :], gmis[:], OP.subtract)
                nc.vector.tensor_add(
                    xm[:], xm[:],
                    bh[:, 0:NRU].unsqueeze(2).to_broadcast([128, NRU, BL]))
            pre = wpool.tile([128, NRU, BL], F32, tag="pre")
            aeng.tensor_tensor(pre[:], tru[:], xm[:], OP.add)
            sig = wpool.tile([128, NRU, BL], F32, tag="sig")
            nc.scalar.activation(
                sig[:].rearrange("p a b -> p (a b)"),
                pre[:].rearrange("p a b -> p (a b)"), AF.Sigmoid)

            y_c, mis_c = group_chain("c", 1, zqc, float(H), NC_, sbp, mbp)
            # c apply
            tc_ = wpool.tile([128, NC_, BL], F32, tag="tc_")
            aeng.tensor_tensor(
                tc_[:], zqc[:, 0, :, :],
                y_c.unsqueeze(1).to_broadcast([128, NC_, BL]), OP.mult)
            oc = wpool.tile([128, NC_, BL], F32, tag="oc")
            aeng.tensor_tensor(
                oc[:], tc_[:],
                mis_c.unsqueeze(1).to_broadcast([128, NC_, BL]),
                OP.subtract)
            if general_ln:
                nc.vector.tensor_mul(
                    oc[:], oc[:],
                    gh[:, NRU:NT].unsqueeze(2).to_broadcast([128, NC_, BL]))
                nc.vector.tensor_add(
                    oc[:], oc[:],
                    bh[:, NRU:NT].unsqueeze(2).to_broadcast([128, NC_, BL]))
            rh = wpool.tile([128, NC_, BL], F32, tag="rh")
            aeng.tensor_tensor(rh[:], sig[:, 0:NC_, :], oc[:], OP.mult)
            prec = wpool.tile([128, NC_, BL], F32, tag="prec")
            aeng.tensor_tensor(prec[:], rh[:], xs[:, NRU:NT, :], OP.add)
            cc = wpool.tile([128, NC_, BL], F32, tag="cc")
            nc.scalar.activation(
                cc[:].rearrange("p a b -> p (a b)"),
                prec[:].rearrange("p a b -> p (a b)"), AF.Tanh)
            dd = wpool.tile([128, KH, BL], F32, tag="dd")
            aeng.tensor_tensor(dd[:], cc[:], h_prev, OP.subtract)
            ud = wpool.tile([128, KH, BL], F32, tag="ud")
            aeng.tensor_tensor(ud[:], sig[:, NC_:NRU, :], dd[:], OP.mult)
            # bf16 h for the next step's matmuls FIRST (it gates the PE),
            # then the fp32 hist/output copy off the critical path
            hb = hpool.tile([128, KH, BL], BF16, tag="hb")
            aeng.tensor_tensor(hb[:], h_prev, ud[:], OP.add)
            aeng.tensor_tensor(h_out, h_prev, ud[:], OP.add)
            return hb

        def flush_block(hist, tb_expr):
            for k in range(KH):
                tp = tpool.tile([128, 128], F32, tag="ftp")
                nc.tensor.transpose(tp[:], hist[:, k, :, :], identity[:])
                if k % 2 == 0:
                    nc.scalar.copy(obuf[:, k, :], tp[:])
                else:
                    nc.vector.tensor_copy(obuf[:, k, :], tp[:])
            nc.sync.dma_start(
                out_d[:, ds(tb_expr, BLK), :].transpose([1, 0, 2]),
                obuf[:].rearrange("p k n -> p (k n)"))

        def _emit_body(ib):
            hb = hpool.tile([128, KH, BL], BF16, tag="hb")
            nc.vector.tensor_copy(hb[:], histQ[:, :, BLK - 1, :])
            nc.sync.dma_start(
                xfB[:],
                zx_d[:, :, ds((ib + CHUNK) * BL, CHUNK * BL)].transpose([1, 0, 2]))
            for half in range(2):
                xf = (xfA, xfB)[half]
                for blk in range(4):
                    gblk = half * 4 + blk
                    hist = (histP, histQ)[gblk % 2]
                    prev_hist = (histP, histQ)[(gblk + 1) % 2]
                    for s in range(BLK):
                        cstep = blk * BLK + s
                        h_prev = (hist[:, :, s - 1, :] if s > 0
                                  else prev_hist[:, :, BLK - 1, :])
                        hb = emit_step(h_prev, hist[:, :, s, :], hb, xf, cstep)
                    flush_block(hist, ib + gblk * BLK)
            nc.sync.dma_start(
                xfA[:],
                zx_d[:, :, ds((ib + 2 * CHUNK) * BL, CHUNK * BL)].transpose([1, 0, 2]))

        if sim_steps is not None:
            for ib2 in range(0, sim_steps, STEPS_PER_BODY):
                _emit_body(ib2)
        else:
            with tc.For_i(0, T, STEPS_PER_BODY,
                          hint_engines=(mybir.EngineType.PE,
                                        mybir.EngineType.DVE,
                                        mybir.EngineType.Activation,
                                        mybir.EngineType.Pool)) as ib:
                _emit_body(ib)

    nc.compile()
    return nc


_CACHE = {}
LAST_RESULT = None


def _get_program(general_ln: bool):
    if general_ln not in _CACHE:
        _CACHE[general_ln] = _build_program(general_ln)
    return _CACHE[general_ln]


def build_in_maps(inputs):
    return _prep(**inputs)[0]


def _prep(x, W_xr, W_xu, W_xc, W_hr, W_hu, W_hc, h0,
          ln_xru_scale, ln_xru_bias, ln_hru_scale, ln_hru_bias,
          ln_xc_scale, ln_xc_bias, ln_hc_scale, ln_hc_bias):
    x = np.ascontiguousarray(np.asarray(x, np.float32))
    wx = np.concatenate([W_xr, W_xu, W_xc], axis=1).astype(np.float32)
    wh = np.concatenate([W_hr, W_hu, W_hc], axis=1).astype(np.float32)
    whb = np.ascontiguousarray(wh.astype(ml_dtypes.bfloat16))
    whf = whb.astype(np.float32)
    whsb = np.stack([whf[:, :2 * H].sum(1) / (2 * H),
                     whf[:, 2 * H:].sum(1) / H], axis=1)
    whsb = np.ascontiguousarray(whsb.astype(ml_dtypes.bfloat16))

    gx_full = np.concatenate([ln_xru_scale, ln_xc_scale]).astype(np.float32)
    bx_full = np.concatenate([ln_xru_bias, ln_xc_bias]).astype(np.float32)
    gh_full = np.concatenate([ln_hru_scale, ln_hc_scale]).astype(np.float32)
    bh_full = np.concatenate([ln_hru_bias, ln_hc_bias]).astype(np.float32)
    general_ln = not (np.all(gx_full == 1) and np.all(bx_full == 0)
                      and np.all(gh_full == 1) and np.all(bh_full == 0))

    h0 = np.asarray(h0, np.float32)
    h0t = np.repeat(h0.reshape(KH, 128).T[:, :, None], BL, axis=2)
    h0t = np.ascontiguousarray(h0t.reshape(128, KH * BL), np.float32)

    ident = np.eye(128, dtype=np.float32)

    shared = {
        "wx": np.ascontiguousarray(wx), "whb": whb, "whsb": whsb,
        "h0t": h0t, "ident": ident,
    }
    if general_ln:
        shared["gx"] = np.broadcast_to(gx_full, (128, H3)).copy()
        shared["bx"] = np.broadcast_to(bx_full, (128, H3)).copy()
        shared["gh"] = np.ascontiguousarray(gh_full.reshape(NT, 128).T)
        shared["bh"] = np.ascontiguousarray(bh_full.reshape(NT, 128).T)

    in_maps = []
    for c in range(NCORES):
        xl = x[c * BL:(c + 1) * BL]                      # [BL, T, D]
        xT = np.ascontiguousarray(
            xl.transpose(2, 1, 0).reshape(D, ROWS), np.float32)
        in_maps.append({"xT": xT, **shared})

    return in_maps, general_ln


def kernel(**inputs):
    in_maps, general_ln = _prep(**inputs)
    nc = _get_program(general_ln)
    res = run_bass_kernel_spmd(nc, in_maps, list(range(NCORES)))
    global LAST_RESULT
    LAST_RESULT = res
    outs = [res.results[c]["out"] for c in range(NCORES)]
    return np.concatenate(outs, axis=0).astype(np.float32)


if __name__ == "__main__":
    rng = np.random.default_rng(0)
    ins = {
        "x": rng.standard_normal((B, T, D), dtype=np.float32),
        "W_xr": rng.standard_normal((D, H), dtype=np.float32) / np.sqrt(D),
        "W_xu": rng.standard_normal((D, H), dtype=np.float32) / np.sqrt(D),
        "W_xc": rng.standard_normal((D, H), dtype=np.float32) / np.sqrt(D),
        "W_hr": rng.standard_normal((H, H), dtype=np.float32) / np.sqrt(H),
        "W_hu": rng.standard_normal((H, H), dtype=np.float32) / np.sqrt(H),
        "W_hc": rng.standard_normal((H, H), dtype=np.float32) / np.sqrt(H),
        "h0": np.zeros(H, np.float32),
        "ln_xru_scale": np.ones(2 * H, np.float32),
        "ln_xru_bias": np.zeros(2 * H, np.float32),
        "ln_hru_scale": np.ones(2 * H, np.float32),
        "ln_hru_bias": np.zeros(2 * H, np.float32),
        "ln_xc_scale": np.ones(H, np.float32),
        "ln_xc_bias": np.zeros(H, np.float32),
        "ln_hc_scale": np.ones(H, np.float32),
        "ln_hc_bias": np.zeros(H, np.float32),
    }
    out = kernel(**ins)
    print(out.shape, out.dtype, np.abs(out).mean())
